# revision 6
# baseline (speedup 1.0000x reference)
"""Trainium2 Bass kernel for nn_CoattentionModel (B,N,T,H,V,C = 256,196,26,512,12000,1000).

Data-parallel: 32 samples per core x 8 cores, weights replicated.
All GEMMs bf16 (fp32 PSUM accumulation). Feature-major activation layout
[128, chunk, cols]. Image col-tiled at 392 (=2*196) so co-attention quads read
whole tiles. v-attention runs with n on the free axis (wide matmul logits,
softmax on vector, weighted sums on the Pool engine).
"""
import numpy as np
import ml_dtypes

import concourse.bacc as bacc
import concourse.mybir as mybir
import concourse.tile as tile
from concourse import bass
from concourse.bass_utils import run_bass_kernel_spmd

B, NI, T, H, V, C = 256, 196, 26, 512, 12000, 1000
NCORES = 8
BL = B // NCORES          # 32
RI = BL * NI              # 6272
KC = 4                    # H/128
SW = 96                   # stacked width per sample (3 sources x 32)
QW = BL * SW              # 3072
WPW = 2 + BL * 32 + 2     # 1028
G4 = 4 * H                # 2048
RQ = BL * T               # 832
TW = 392                  # image col tile width (2 samples' worth of N)
NT = RI // TW             # 16 tiles

P = 128
F32 = mybir.dt.float32
F32R = mybir.dt.float32r
BF16 = mybir.dt.bfloat16
I32 = mybir.dt.int32
AF = mybir.ActivationFunctionType
ALU = mybir.AluOpType
AX = mybir.AxisListType
BF_NP = ml_dtypes.bfloat16

DEBUG_TAPS = ()


def _w4(w):  # [512, F] -> [128, 4, F] bf16
    return np.ascontiguousarray(
        np.asarray(w, np.float32).reshape(KC, P, -1).transpose(1, 0, 2)).astype(BF_NP)


def _b4(b, nch=KC):  # [nch*128] -> [128, nch] f32
    return np.ascontiguousarray(np.asarray(b, np.float32).reshape(nch, P).T)


def _bcast(b):  # [F] -> [128, F] bf16 broadcast over partitions
    return np.ascontiguousarray(
        np.broadcast_to(np.asarray(b, np.float32).reshape(1, -1), (P, len(np.asarray(b).reshape(-1))))
    ).astype(BF_NP)


def host_prep(inputs):
    f = {k: np.asarray(v) for k, v in inputs.items()}
    sh = {}
    sh["embed_table"] = np.ascontiguousarray(f["embed_table"], np.float32)
    for nm in ("W_img", "W_corr", "W_ie", "W_qe", "W_w"):
        sh[nm] = _w4(f[nm])
    sh["W_uni"] = _w4(f["W_uni"][0])
    sh["W_bi"] = np.stack([_w4(f["W_bi"][i]) for i in range(2)], 1)
    sh["W_tri"] = np.stack([_w4(f["W_tri"][i]) for i in range(3)], 1)
    perm = np.concatenate([np.arange(0, H), np.arange(H, 2 * H),
                           np.arange(3 * H, 4 * H), np.arange(2 * H, 3 * H)])
    sh["lstm_k"] = _w4(f["lstm_k"][:, perm])
    sh["lstm_rk"] = _w4(f["lstm_rk"][:, perm])
    sh["lstmb_row"] = f["lstm_b"][perm].astype(BF_NP).reshape(1, G4)
    sh["W_p"] = np.ascontiguousarray(
        f["W_p"].astype(np.float32).reshape(8, P, H).transpose(1, 0, 2)).astype(BF_NP)
    sh["W_s"] = np.ascontiguousarray(
        f["W_s"].astype(np.float32).reshape(8, P, H).transpose(1, 0, 2)).astype(BF_NP)
    sh["W_fc1"] = _w4(f["W_fc1"])
    sh["W_fc"] = np.ascontiguousarray(
        f["W_fc"].astype(np.float32).reshape(8, P, C).transpose(1, 0, 2)).astype(BF_NP)
    sh["w_qa"] = np.ascontiguousarray(_b4(f["w_qa"].reshape(-1)).astype(BF_NP))
    sh["w_ia"] = np.ascontiguousarray(_b4(f["w_ia"].reshape(-1)).astype(BF_NP))
    for nm in ("b_img", "b_corr", "b_ie", "b_qe", "b_uni", "b_bi", "b_tri",
               "b_w", "b_p", "b_s"):
        sh[nm] = _b4(f[nm])
    sh["b_fc1"] = _b4(f["b_fc1"], 8)
    sh["bie_row"] = f["b_ie"].astype(BF_NP).reshape(1, H)
    sh["bqe_row"] = f["b_qe"].astype(BF_NP).reshape(1, H)
    sh["b_fc_row"] = f["b_fc"].astype(BF_NP).reshape(1, C)
    sh["b_qa_col"] = np.full((1, 1), np.asarray(f["b_qa"]).reshape(-1)[0], np.float32)
    sh["b_ia_col"] = np.full((1, 1), np.asarray(f["b_ia"]).reshape(-1)[0], np.float32)
    sh["identity"] = np.eye(P, dtype=np.float32)
    sh["identity_bf"] = np.eye(P, dtype=BF_NP)
    sh["ones_bf"] = np.ones((P, 512), BF_NP)
    sh["zpad"] = np.zeros((P, KC, 2), np.float32)
    sh["zpad_bf"] = np.zeros((P, KC, 256), BF_NP)

    img = f["image"].astype(np.float32)
    qn = np.asarray(f["question"])
    percore = []
    for ci in range(NCORES):
        iloc = img[ci * BL:(ci + 1) * BL].reshape(RI, H)
        imageT = iloc.T.reshape(KC, P, RI).transpose(1, 0, 2)  # [P, KC, RI]
        imageT = np.ascontiguousarray(
            imageT.reshape(P, KC, NT, TW).transpose(2, 0, 1, 3)).astype(BF_NP)
        qi = np.full((BL, 32), V, np.int32)
        qi[:, :T] = qn[ci * BL:(ci + 1) * BL]
        qidx = np.ascontiguousarray(qi.reshape(8, P).T).astype(np.int32)
        percore.append({"imageT": imageT, "qidx": qidx})
    return sh, percore


# ----------------------------------------------------------------------------
def build_nc():
    nc = bacc.Bacc(name="coattn", num_swdge_queues=4)
    D = {}

    def dp(n, s, d=BF16):
        D[n] = nc.declare_dram_parameter(n, list(s), d, isOutput=False)

    dp("embed_table", [V, H], F32)
    for n in ("W_img", "W_corr", "W_ie", "W_qe", "W_w", "W_uni"):
        dp(n, [P, KC, H])
    dp("W_bi", [P, 2, KC, H])
    dp("W_tri", [P, 3, KC, H])
    dp("lstm_k", [P, KC, G4])
    dp("lstm_rk", [P, KC, G4])
    dp("lstmb_row", [1, G4])
    dp("W_p", [P, 8, H])
    dp("W_s", [P, 8, H])
    dp("W_fc1", [P, KC, 2 * H])
    dp("W_fc", [P, 8, C])
    dp("w_qa", [P, KC])
    dp("w_ia", [P, KC])
    for n in ("b_img", "b_corr", "b_ie", "b_qe", "b_uni", "b_bi", "b_tri",
              "b_w", "b_p", "b_s"):
        dp(n, [P, KC], F32)
    dp("b_fc1", [P, 8], F32)
    dp("bie_row", [1, H])
    dp("bqe_row", [1, H])
    dp("b_fc_row", [1, C])
    dp("b_qa_col", [1, 1], F32)
    dp("b_ia_col", [1, 1], F32)
    dp("identity", [P, P], F32R)
    dp("identity_bf", [P, P])
    dp("ones_bf", [P, 512])
    dp("zpad", [P, KC, 2], F32R)
    dp("zpad_bf", [P, KC, 256])
    dp("imageT", [NT, P, KC, TW])
    dp("qidx", [P, 8], I32)
    D["yout"] = nc.declare_dram_parameter("yout", [BL, C], F32, isOutput=True)

    D["corrT_d"] = nc.dram_tensor("corrT_d", [NT, P, KC, TW], BF16)
    D["embT_d"] = nc.dram_tensor("embT_d", [NT, P, KC, TW], BF16)
    D["rowemb_d"] = nc.dram_tensor("rowemb_d", [RI, H], BF16)
    D["qrow_d"] = nc.dram_tensor("qrow_d", [BL, P, H], BF16)
    D["xp_d"] = nc.dram_tensor("xp_d", [RQ, G4], BF16)

    def tap(name, ap, shape, dtype):
        if name in DEBUG_TAPS:
            o = nc.declare_dram_parameter("tap_" + name, list(shape), dtype, isOutput=True)
            nc.sync.dma_start(out=o[:], in_=ap)

    with tile.TileContext(nc) as tc:
        import contextlib
        with contextlib.ExitStack() as es:
            _build(nc, tc, D, tap, es)
    nc.compile()
    return nc


def _build(nc, tc, D, tap, es):
    import os as _os
    PH = int(_os.environ.get("KER_PHASES", "99"))
    import contextlib
    es_words = contextlib.ExitStack()
    es_phrase = contextlib.ExitStack()
    es_lk = contextlib.ExitStack()
    es_xp = contextlib.ExitStack()
    es_lstm = contextlib.ExitStack()
    es_sent = contextlib.ExitStack()
    es_qe = contextlib.ExitStack()
    const = es.enter_context(tc.tile_pool(name="const", bufs=1))
    resid = es.enter_context(tc.tile_pool(name="resid", bufs=1))

    def load_const(name, shape, dtype):
        t_ = const.tile(shape, dtype, tag=name, name="c_" + name)
        nc.sync.dma_start(out=t_[:], in_=D[name][:])
        return t_

    identR = load_const("identity", [P, P], F32R)
    identB = load_const("identity_bf", [P, P], BF16)
    ones_bf = load_const("ones_bf", [P, 512], BF16)
    zp = load_const("zpad", [P, KC, 2], F32R)
    zpb = load_const("zpad_bf", [P, KC, 256], BF16)
    bias = {n: load_const(n, [P, KC], F32) for n in
            ("b_img", "b_corr", "b_ie", "b_qe", "b_uni", "b_bi", "b_tri",
             "b_w", "b_p", "b_s")}
    bias["b_fc1"] = load_const("b_fc1", [P, 8], F32)
    bie_row = load_const("bie_row", [1, H], BF16)
    bqe_row = load_const("bqe_row", [1, H], BF16)
    wqa = load_const("w_qa", [P, KC], BF16)
    wia = load_const("w_ia", [P, KC], BF16)
    bqa_col = load_const("b_qa_col", [1, 1], F32)
    bia_col = load_const("b_ia_col", [1, 1], F32)

    # image-phase pools created BEFORE lstm-era pools so their SBUF/PSUM
    # addresses don't alias (address-reuse would serialize image behind LSTM)
    es_img = contextlib.ExitStack()
    iw = es_img.enter_context(tc.tile_pool(name="imgw", bufs=1))
    io2 = es_img.enter_context(tc.tile_pool(name="imgio2", bufs=2))
    io1 = es_img.enter_context(tc.tile_pool(name="imgio1", bufs=2))
    psa = es_img.enter_context(tc.tile_pool(name="psA", bufs=3, space="PSUM"))
    wimg = iw.tile([P, KC, H], BF16)
    nc.sync.dma_start(out=wimg[:], in_=D["W_img"][:])
    wcorr = iw.tile([P, KC, H], BF16)
    nc.sync.dma_start(out=wcorr[:], in_=D["W_corr"][:])
    wie = iw.tile([P, KC, H], BF16)
    nc.sync.dma_start(out=wie[:], in_=D["W_ie"][:])

    # long-lived phase pools, created in reverse order of release (LIFO)
    qepool = es_qe.enter_context(tc.tile_pool(name="qew", bufs=1))
    wqe = qepool.tile([P, KC, H], BF16)
    nc.sync.dma_start(out=wqe[:], in_=D["W_qe"][:])
    spool = es_sent.enter_context(tc.tile_pool(name="sentp", bufs=1))
    sentT = spool.tile([P, KC, RQ], BF16)
    xpool = es_xp.enter_context(tc.tile_pool(name="xpp", bufs=2))
    ppool = es_phrase.enter_context(tc.tile_pool(name="phrasep", bufs=1))
    phraseT = ppool.tile([P, KC, RQ], BF16)

    stk = resid.tile([P, KC, QW + 32], BF16)
    qembT = resid.tile([P, KC, QW], BF16)
    qfeat = resid.tile([P, KC, 3 * BL], F32)
    vfeat = resid.tile([P, KC, 3 * BL], F32)
    itf = resid.tile([P, KC, RI], BF16)      # tanh(image@W_img), feature-major
    stk_b = stk[:, :, :QW].rearrange("p k (b w) -> p k b w", w=SW)
    for c0 in range(3):
        for k in range(KC):
            nc.sync.dma_start(
                out=stk_b[:, k, :, 32 * c0 + 26:32 * c0 + 32],
                in_=zpb[:, k, :BL * 6].rearrange("p (b w) -> p b w", w=6))
    nc.sync.dma_start(out=stk[:, :, QW:], in_=zpb[:, :, :32])

    # ================= phase 0: embedding gather =================
    wpool = es_words.enter_context(tc.tile_pool(name="wordsp", bufs=1))
    wordsT = wpool.tile([P, KC, WPW], BF16)
    nc.sync.dma_start(out=wordsT[:, :, :2], in_=zpb[:, :, :2])
    nc.sync.dma_start(out=wordsT[:, :, WPW - 2:], in_=zpb[:, :, :2])
    qix = const.tile([P, 8], I32, tag="qix")
    nc.sync.dma_start(out=qix[:], in_=D["qidx"][:])
    with (tc.tile_pool(name="gath", bufs=4) as gp,
          tc.tile_pool(name="psG0", bufs=2, space="PSUM") as psg):
        for j in range(8):
            gt = gp.tile([P, H], F32R, tag="g")
            nc.vector.memset(gt[:].bitcast(F32), 0.0)
            nc.gpsimd.indirect_dma_start(
                out=gt[:], out_offset=None, in_=D["embed_table"][:],
                in_offset=bass.IndirectOffsetOnAxis(ap=qix[:, j:j + 1], axis=0),
                bounds_check=V - 1, oob_is_err=False)
            pt = psg.tile([P, H], F32R, tag="t")
            for k in range(KC):
                nc.tensor.transpose(out=pt[:, k * P:(k + 1) * P],
                                    in_=gt[:, k * P:(k + 1) * P], identity=identR[:])
            nc.scalar.copy(wordsT[:, :, 2 + j * P:2 + (j + 1) * P],
                           pt[:].rearrange("p (k c) -> p k c", c=P))
    wvalid = wordsT[:, :, 2:2 + BL * 32].rearrange("p k (b w) -> p k b w", w=32)[:, :, :, :T]
    nc.scalar.copy(stk_b[:, :, :, 0:T], wvalid)

    if PH < 1:
        return
    # ================= phase 1: convs -> phrase =================
    with (tc.tile_pool(name="convw", bufs=1) as cw,
          tc.tile_pool(name="convo", bufs=1) as co,
          tc.tile_pool(name="psC", bufs=3, space="PSUM") as psc):
        mx = co.tile([P, KC, RQ], BF16, tag="mx")
        cv1 = co.tile([P, KC, RQ], BF16, tag="cv1")
        cvs = [mx, cv1, cv1]
        taps_w = [
            ("b_uni", [(0, D["W_uni"][:])]),
            ("b_bi", [(-1, D["W_bi"][:, 0]), (1, D["W_bi"][:, 1])]),
            ("b_tri", [(-2, D["W_tri"][:, 0]), (0, D["W_tri"][:, 1]),
                       (2, D["W_tri"][:, 2])]),
        ]
        for ci, (bn, tp) in enumerate(taps_w):
            wts = []
            for ti, (shf, src) in enumerate(tp):
                wct = cw.tile([P, KC, H], BF16, tag=f"cw{ti}", name=f"cw{ti}")
                nc.sync.dma_start(out=wct[:], in_=src)
                wts.append((shf, wct))
            tp = [(shf, (lambda w_: lambda k, m: w_[:, k, m * P:(m + 1) * P])(w_))
                  for shf, w_ in wts]
            for hf in range(2):
                for m in range(KC):
                    pt = psc.tile([P, 16, T], F32, tag="c")
                    n_mm = len(tp) * KC
                    i_mm = 0
                    for shf, wfn in tp:
                        for k in range(KC):
                            rhs = wordsT[:, k, 2 + hf * 512 + shf:2 + hf * 512 + shf + 512] \
                                .rearrange("p (b w) -> p b w", w=32)[:, :, :T]
                            nc.tensor.matmul(out=pt[:], lhsT=wfn(k, m), rhs=rhs,
                                             start=(i_mm == 0), stop=(i_mm == n_mm - 1))
                            i_mm += 1
                    nc.scalar.activation(
                        cvs[ci][:, m, hf * 416:(hf + 1) * 416]
                        .rearrange("p (b w) -> p b w", w=T),
                        pt[:], AF.Tanh, bias=bias[bn][:, m:m + 1], scale=1.0)
                    if ci > 0:
                        nc.vector.tensor_tensor(
                            out=mx[:, m, hf * 416:(hf + 1) * 416],
                            in0=mx[:, m, hf * 416:(hf + 1) * 416],
                            in1=cv1[:, m, hf * 416:(hf + 1) * 416], op=ALU.max)
        nc.scalar.activation(phraseT[:], mx[:], AF.Tanh)
        nc.scalar.copy(stk_b[:, :, :, 32:32 + T],
                       phraseT.rearrange("p k (b t) -> p k b t", t=T))
    tap("stk", stk[:], [P, KC, QW + 32], BF16)

    # ================= phase 2: q_emb feature-major (3 sources) ==========
    if PH < 2:
        return
    def qemb_from(src_fn, block):
        # src_fn(k, hf) -> rhs AP [128, 16, 26] for batch-half hf, chunk k
        with tc.tile_pool(name=f"psQE{block}", bufs=2, space="PSUM") as psq_:
            for m in range(KC):
                for hf in range(2):
                    pt = psq_.tile([P, 16, T], F32, tag="q")
                    for k in range(KC):
                        nc.tensor.matmul(out=pt[:], lhsT=wqe[:, k, m * P:(m + 1) * P],
                                         rhs=src_fn(k, hf),
                                         start=(k == 0), stop=(k == KC - 1))
                    nc.vector.tensor_scalar_add(
                        qembT.rearrange("p k (b w) -> p k b w", w=SW)
                        [:, m, hf * 16:(hf + 1) * 16, 32 * block:32 * block + T],
                        pt[:], bias["b_qe"][:, m:m + 1])

    qemb_from(lambda k, hf: wordsT[:, k, 2 + hf * 512:2 + (hf + 1) * 512]
              .rearrange("p (b w) -> p b w", w=32)[:, :, :T], 0)
    es_words.close()
    qemb_from(lambda k, hf: phraseT[:, k, hf * 416:(hf + 1) * 416]
              .rearrange("p (b t) -> p b t", t=T), 1)

    if PH < 3:
        return
    # ================= phase 3: xp GEMM =================
    lkp = es_lk.enter_context(tc.tile_pool(name="lkp", bufs=2))
    lkc = es_lk.enter_context(tc.tile_pool(name="lkc", bufs=1))
    lstmb_row = lkc.tile([1, G4], BF16)
    nc.sync.dma_start(out=lstmb_row[:], in_=D["lstmb_row"][:])

    with tc.tile_pool(name="psX", bufs=2, space="PSUM") as psx:
        for j in range(KC):
            lk = lkp.tile([P, KC, 512], BF16, tag="lk")
            nc.sync.dma_start(out=lk[:], in_=D["lstm_k"][:, :, j * 512:(j + 1) * 512])
            for rc in range(7):           # b-major rows b*26+t, chunks of 128
                nr = 128 if rc < 6 else 64
                xstg = xpool.tile([P, 512], BF16, tag="xstg")
                pt = psx.tile([P, 512], F32, tag="x")
                nc.tensor.matmul(out=pt[:nr, :], lhsT=ones_bf[0:1, :nr],
                                 rhs=lstmb_row[0:1, j * 512:(j + 1) * 512],
                                 start=True, stop=False)
                for k in range(KC):
                    nc.tensor.matmul(out=pt[:nr, :],
                                     lhsT=phraseT[:, k, rc * P:rc * P + nr],
                                     rhs=lk[:, k, :],
                                     start=False, stop=(k == KC - 1))
                nc.scalar.copy(xstg[:nr, :], pt[:nr, :])
                nc.sync.dma_start(
                    out=D["xp_d"][rc * P:rc * P + nr, j * 512:(j + 1) * 512],
                    in_=xstg[:nr, :])
    tap("xp", D["xp_d"][:], [RQ, G4], BF16)
    es_lk.close()
    es_phrase.close()

    if PH < 4:
        return
    # ================= phase 4: LSTM =================
    lrkp = es_lstm.enter_context(tc.tile_pool(name="lrkp", bufs=1))
    lrk = lrkp.tile([P, KC, G4], BF16)
    nc.sync.dma_start(out=lrk[:], in_=D["lstm_rk"][:])
    lpool = es_lstm.enter_context(tc.tile_pool(name="lstm", bufs=1))
    gs = lpool.tile([32, G4], F32)
    cst = lpool.tile([32, H], F32)
    tct = lpool.tile([32, H], F32)
    hrow = lpool.tile([32, H], BF16)
    hT_pool = es_lstm.enter_context(tc.tile_pool(name="hT", bufs=2))
    psg_l = es_lstm.enter_context(tc.tile_pool(name="psGate", bufs=3, space="PSUM"))
    psh_l = es_lstm.enter_context(tc.tile_pool(name="psH", bufs=2, space="PSUM"))
    hT_prev = None
    for t_ in range(T):
        xpt = hT_pool.tile([32, G4], BF16, tag="xpt")
        nc.sync.dma_start(out=xpt[:],
                          in_=D["xp_d"][:].rearrange("(b t) g -> t b g", t=T)[t_])
        for j in range(KC):
            pg = psg_l.tile([32, 512], F32, tag="g")
            nc.tensor.matmul(out=pg[:], lhsT=identB[0:32, 0:32],
                             rhs=xpt[:, j * 512:(j + 1) * 512],
                             start=True, stop=(hT_prev is None))
            if hT_prev is not None:
                for k in range(KC):
                    nc.tensor.matmul(out=pg[:],
                                     lhsT=hT_prev[:, k, :],
                                     rhs=lrk[:, k, j * 512:(j + 1) * 512],
                                     start=False, stop=(k == KC - 1))
            nc.scalar.activation(gs[:, j * 512:(j + 1) * 512], pg[:],
                                 AF.Sigmoid if j < 3 else AF.Tanh)
        if t_ == 0:
            nc.vector.tensor_tensor(out=cst[:], in0=gs[:, 0:512],
                                    in1=gs[:, 1536:2048], op=ALU.mult)
        else:
            nc.gpsimd.tensor_tensor(out=tct[:], in0=gs[:, 512:1024], in1=cst[:],
                                    op=ALU.mult)
            nc.vector.tensor_tensor(out=cst[:], in0=gs[:, 0:512],
                                    in1=gs[:, 1536:2048], op=ALU.mult)
            nc.vector.tensor_tensor(out=cst[:], in0=cst[:], in1=tct[:], op=ALU.add)
        nc.scalar.activation(tct[:], cst[:], AF.Tanh)
        nc.vector.tensor_tensor(out=hrow[:], in0=gs[:, 1024:1536], in1=tct[:],
                                op=ALU.mult)
        ph = psh_l.tile([P, P], BF16, tag="h")
        for k in range(KC):
            nc.tensor.transpose(out=ph[:, 32 * k:32 * (k + 1)],
                                in_=hrow[:, k * P:(k + 1) * P],
                                identity=identB[:32, :32])
        hT = hT_pool.tile([P, KC, 32], BF16, tag="hT")
        nc.scalar.copy(hT[:], ph[:].rearrange("p (k b) -> p k b", b=32))
        nc.scalar.copy(sentT.rearrange("p k (b t) -> p k b t", t=T)[:, :, :, t_], hT[:])
        hT_prev = hT
    tap("sentT", sentT[:], [P, KC, RQ], BF16)
    es_lstm.close()
    es_xp.close()

    if PH < 5:
        return
    # ============ phase 5: q_emb[sentence], stk[sentence], qrow ============
    nc.scalar.copy(stk_b[:, :, :, 64:64 + T],
                   sentT.rearrange("p k (b t) -> p k b t", t=T))
    qemb_from(lambda k, hf: sentT[:, k, hf * 416:(hf + 1) * 416]
              .rearrange("p (b t) -> p b t", t=T), 2)
    tap("qembT", qembT[:], [P, KC, QW], BF16)
    es_sent.close()

    with (tc.tile_pool(name="qrstage", bufs=2) as qrs,
          tc.tile_pool(name="psQR", bufs=2, space="PSUM") as psqr):
        for b in range(BL):
            pt = psqr.tile([96, H], F32, tag="r")
            nc.tensor.matmul(out=pt[:], lhsT=ones_bf[0:1, :96],
                             rhs=bqe_row[0:1, :], start=True, stop=False)
            for k in range(KC):
                nc.tensor.matmul(out=pt[:], lhsT=stk[:, k, b * SW:(b + 1) * SW],
                                 rhs=wqe[:, k, :], start=False, stop=(k == KC - 1))
            st = qrs.tile([96, H], BF16, tag="s")
            nc.vector.tensor_copy(st[:], pt[:])
            nc.sync.dma_start(out=D["qrow_d"][b, :96, :], in_=st[:])
    es_qe.close()

    if PH < 6:
        return
    # ================= phase 6: image GEMMs =================
    for ti in range(NT):
        ait = io2.tile([P, KC, TW], BF16, tag="ait")
        nc.sync.dma_start(out=ait[:], in_=D["imageT"][ti])
        for m in range(KC):
            pt = psa.tile([P, 512], F32, tag="m")
            for k in range(KC):
                nc.tensor.matmul(out=pt[:, :TW], lhsT=wimg[:, k, m * P:(m + 1) * P],
                                 rhs=ait[:, k, :], start=(k == 0), stop=(k == KC - 1))
            nc.scalar.activation(itf[:, m, ti * TW:(ti + 1) * TW], pt[:, :TW],
                                 AF.Tanh, bias=bias["b_img"][:, m:m + 1], scale=1.0)
        for wmat, bn, dst in ((wcorr, "b_corr", D["corrT_d"]),
                              (wie, "b_ie", D["embT_d"])):
            stg = io1.tile([P, KC, TW], BF16, tag="stg" + bn, name="stg" + bn)
            for m in range(KC):
                pt = psa.tile([P, 512], F32, tag="m")
                for k in range(KC):
                    nc.tensor.matmul(out=pt[:, :TW],
                                     lhsT=wmat[:, k, m * P:(m + 1) * P],
                                     rhs=itf[:, k, ti * TW:(ti + 1) * TW],
                                     start=(k == 0), stop=(k == KC - 1))
                nc.vector.tensor_scalar_add(stg[:, m, :], pt[:, :TW],
                                            bias[bn][:, m:m + 1])
            nc.sync.dma_start(out=dst[ti], in_=stg[:])
    # row-major img_embed for the ques_sum matmuls
    for rc in range(RI // P):
        pt = psa.tile([P, 512], F32, tag="m")
        nc.tensor.matmul(out=pt[:], lhsT=ones_bf[0:1, :P], rhs=bie_row[0:1, :],
                         start=True, stop=False)
        for k in range(KC):
            nc.tensor.matmul(out=pt[:], lhsT=itf[:, k, rc * P:(rc + 1) * P],
                             rhs=wie[:, k, :], start=False, stop=(k == KC - 1))
        rstg = io1.tile([P, H], BF16, tag="rstg")
        nc.scalar.copy(rstg[:], pt[:])
        nc.sync.dma_start(out=D["rowemb_d"][rc * P:(rc + 1) * P, :], in_=rstg[:])
    es_img.close()
    tap("corrT", D["corrT_d"][:], [NT, P, KC, TW], BF16)
    tap("rowemb", D["rowemb_d"][:], [RI, H], BF16)

    if PH < 7:
        return
    # ================= phase 7: co-attention =================
    with (tc.tile_pool(name="castream", bufs=1) as cs,
          tc.tile_pool(name="castream2", bufs=2) as cs2,
          tc.tile_pool(name="camid", bufs=2) as cm,
          tc.tile_pool(name="cis", bufs=1) as cis,
          tc.tile_pool(name="caq", bufs=2) as cq,
          tc.tile_pool(name="cabig", bufs=1) as cbig,
          tc.tile_pool(name="psQ", bufs=2, space="PSUM") as psq,
          tc.tile_pool(name="psT", bufs=1, space="PSUM") as pst,
          tc.tile_pool(name="psIS", bufs=2, space="PSUM") as psis,
          tc.tile_pool(name="psV", bufs=2, space="PSUM") as psv):
        for qd in range(8):
            b0 = qd * 4
            corr_q = cs2.tile([P, KC, 4 * NI], BF16, tag="corr")
            emb_q = cs2.tile([P, KC, 4 * NI], BF16, tag="emb")
            for hq in range(2):
                nc.sync.dma_start(out=corr_q[:, :, hq * TW:(hq + 1) * TW],
                                  in_=D["corrT_d"][2 * qd + hq])
                nc.sync.dma_start(out=emb_q[:, :, hq * TW:(hq + 1) * TW],
                                  in_=D["embT_d"][2 * qd + hq])
            itq = itf[:, :, qd * 4 * NI:(qd + 1) * 4 * NI]
            qr = []
            rowe = []
            for s in range(4):
                b = b0 + s
                qr_ = cs.tile([96, H], BF16, tag=f"qr{s}", name=f"qr{s}")
                nc.sync.dma_start(out=qr_[:], in_=D["qrow_d"][b, :96, :])
                qr.append(qr_)
                re_ = cs.tile([P, 2, H], BF16, tag=f"re{s}", name=f"re{s}")
                nc.sync.dma_start(out=re_[:, 0, :], in_=D["rowemb_d"][b * NI:b * NI + P, :])
                nc.sync.dma_start(out=re_[:68, 1, :],
                                  in_=D["rowemb_d"][b * NI + P:(b + 1) * NI, :])
                rowe.append(re_)
            # --- wmT [196p, 96]/sample, quad-wide psums
            pw0 = psq.tile([P, 4 * SW], F32, tag="q")
            pw1 = psq.tile([68, 4 * SW], F32, tag="q")
            for s in range(4):
                b = b0 + s
                for hh, pw, pn in ((0, pw0, P), (1, pw1, 68)):
                    for k in range(KC):
                        nc.tensor.matmul(
                            out=pw[:pn, s * SW:(s + 1) * SW],
                            lhsT=corr_q[:, k, s * NI + hh * P:s * NI + hh * P + pn],
                            rhs=stk[:, k, b * SW:(b + 1) * SW],
                            start=(k == 0), stop=(k == KC - 1))
            wmT = cm.tile([P, 2, 4 * SW], BF16, tag="wmT")
            nc.scalar.activation(wmT[:, 0, :], pw0[:], AF.Tanh)
            nc.scalar.activation(wmT[:68, 1, :], pw1[:], AF.Tanh)
            # --- wm [96p(32c+t), 196]/sample via PE transpose
            wm = cm.tile([96, 4, NI], BF16, tag="wm")
            for pr in range(2):
                ptr = pst.tile([96, 2 * NI], BF16, tag="t")
                for si in range(2):
                    s = pr * 2 + si
                    for hh, pn in ((0, P), (1, 68)):
                        nc.tensor.transpose(
                            out=ptr[:, si * NI + hh * P:si * NI + hh * P + pn],
                            in_=wmT[:pn, hh, s * SW:(s + 1) * SW],
                            identity=identB[:pn, :pn])
                nc.vector.tensor_copy(wm[:, 2 * pr:2 * pr + 2, :],
                                      ptr[:].rearrange("p (s n) -> p s n", n=NI))
            # --- ques_sum feature-major (quad-wide)
            qsT = cm.tile([P, KC, 4 * SW], BF16, tag="qsT")
            for m in range(KC):
                pqs = psq.tile([P, 4 * SW], F32, tag="q")
                for s in range(4):
                    nc.tensor.matmul(out=pqs[:, s * SW:(s + 1) * SW],
                                     lhsT=rowe[s][:, 0, m * P:(m + 1) * P],
                                     rhs=wmT[:, 0, s * SW:(s + 1) * SW],
                                     start=True, stop=False)
                    nc.tensor.matmul(out=pqs[:, s * SW:(s + 1) * SW],
                                     lhsT=rowe[s][:68, 1, m * P:(m + 1) * P],
                                     rhs=wmT[:68, 1, s * SW:(s + 1) * SW],
                                     start=False, stop=True)
                nc.vector.tensor_tensor(out=pqs[:], in0=pqs[:],
                                        in1=qembT[:, m, b0 * SW:(b0 + 4) * SW],
                                        op=ALU.add)
                nc.scalar.activation(qsT[:, m, :], pqs[:], AF.Tanh)
            if qd == 0:
                tap("qsT0", qsT[:], [P, KC, 4 * SW], BF16)
            # --- q attention + q_feat
            pql = psv.tile([P, 512], F32, tag="v")
            for k in range(KC):
                nc.tensor.matmul(out=pql[0:1, 0:4 * SW], lhsT=wqa[:, k:k + 1],
                                 rhs=qsT[:, k, :], start=(k == 0), stop=(k == KC - 1))
            eq = cq.tile([1, 4 * SW], F32, tag="eq")
            nc.scalar.activation(eq[:], pql[0:1, 0:4 * SW], AF.Exp,
                                 bias=bqa_col[:, 0:1], scale=1.0)
            sq = cq.tile([1, 12], F32, tag="sq")
            nc.vector.reduce_sum(sq[:], eq.rearrange("o (s w) -> o s w", w=32)[:, :, :T],
                                 axis=AX.X)
            rq = cq.tile([1, 12], F32, tag="rq")
            nc.vector.reciprocal(rq[:], sq[:])
            for seg in range(12):
                nc.gpsimd.tensor_scalar_mul(eq[0:1, seg * 32:seg * 32 + T],
                                            eq[0:1, seg * 32:seg * 32 + T],
                                            rq[0:1, seg:seg + 1])
            eqb = cq.tile([1, 4 * SW], BF16, tag="eqb")
            nc.scalar.copy(eqb[:], eq[:])
            pqb = psv.tile([P, 512], F32, tag="v")
            nc.tensor.matmul(out=pqb[:, 0:4 * SW], lhsT=ones_bf[0:1, :P], rhs=eqb[:],
                             start=True, stop=True)
            for m in range(KC):
                tq = cq.tile([P, 4 * SW], F32, tag="tq")
                nc.vector.tensor_tensor(out=tq[:], in0=stk[:, m, b0 * SW:(b0 + 4) * SW],
                                        in1=pqb[:, 0:4 * SW], op=ALU.mult)
                nc.vector.reduce_sum(qfeat[:, m, b0 * 3:(b0 + 4) * 3],
                                     tq.rearrange("p (s w) -> p s w", w=32)[:, :, :T],
                                     axis=AX.X)
            # --- img_sum feature-major; v logits with n on the free axis
            isT = cis.tile([P, KC, 3, 4 * NI], BF16, tag="isT")
            for c in range(3):
                for m in range(KC):
                    for half in range(2):
                        pis = psis.tile([P, TW], F32, tag="is")
                        nc.tensor.matmul(
                            out=pis[:], lhsT=identB[:],
                            rhs=emb_q[:, m, half * TW:(half + 1) * TW],
                            start=True, stop=False)
                        for si in range(2):
                            s = half * 2 + si
                            nc.tensor.matmul(
                                out=pis[:, si * NI:(si + 1) * NI],
                                lhsT=qr[s][32 * c:32 * c + T, m * P:(m + 1) * P],
                                rhs=wm[32 * c:32 * c + T, s, :],
                                start=False, stop=(si == 1), tile_position=(32 * c, 0))
                        nc.scalar.activation(
                            isT[:, m, c, half * TW:(half + 1) * TW], pis[:], AF.Tanh)
            # logits [1, n] per (c, half); softmax along free axis
            ev = cbig.tile([1, 3, 4 * NI], F32, tag="ev")
            for c in range(3):
                for half in range(2):
                    plg = psv.tile([P, 512], F32, tag="v")
                    for k in range(KC):
                        nc.tensor.matmul(out=plg[0:1, 0:TW], lhsT=wia[:, k:k + 1],
                                         rhs=isT[:, k, c, half * TW:(half + 1) * TW],
                                         start=(k == 0), stop=(k == KC - 1))
                    nc.scalar.activation(ev[:, c, half * TW:(half + 1) * TW],
                                         plg[0:1, 0:TW], AF.Exp,
                                         bias=bia_col[:, 0:1], scale=1.0)
            sv = cq.tile([1, 12], F32, tag="sv")
            nc.vector.reduce_sum(sv[:], ev.rearrange("o c (s n) -> o (c s) n", n=NI),
                                 axis=AX.X)
            rv = cq.tile([1, 12], F32, tag="rv")
            nc.vector.reciprocal(rv[:], sv[:])
            for c in range(3):
                for s in range(4):
                    nc.gpsimd.tensor_scalar_mul(
                        ev[0:1, c, s * NI:(s + 1) * NI],
                        ev[0:1, c, s * NI:(s + 1) * NI],
                        rv[0:1, c * 4 + s:c * 4 + s + 1])
            evb = cbig.tile([1, 3, 4 * NI], BF16, tag="evb")
            nc.scalar.copy(evb[:], ev[:])
            attB = cbig.tile([P, 3, 4 * NI], BF16, tag="attB")
            for c in range(3):
                for half in range(2):
                    pab = psv.tile([P, 512], F32, tag="v")
                    nc.tensor.matmul(out=pab[:, 0:TW], lhsT=ones_bf[0:1, :P],
                                     rhs=evb[0:1, c, half * TW:(half + 1) * TW],
                                     start=True, stop=True)
                    nc.scalar.copy(attB[:, c, half * TW:(half + 1) * TW],
                                   pab[:, 0:TW])
            # v_feat: att-weighted sums of tanh-image, on the Pool engine
            for k in range(KC):
                for c in range(3):
                    vt = cq.tile([P, 4 * NI], BF16, tag="vt")
                    nc.vector.tensor_tensor(out=vt[:], in0=itq[:, k, :],
                                            in1=attB[:, c, :], op=ALU.mult)
                    nc.vector.reduce_sum(
                        vfeat[:, k, b0 * 3:(b0 + 4) * 3]
                        .rearrange("p (s c) -> p s c", c=3)[:, :, c],
                        vt.rearrange("p (s n) -> p s n", n=NI), axis=AX.X)
    tap("qfeat", qfeat[:], [P, KC, 3 * BL], F32)
    tap("vfeat", vfeat[:], [P, KC, 3 * BL], F32)

    if PH < 8:
        return
    # ================= phase 8: final MLP + softmax =================
    with (tc.tile_pool(name="finw", bufs=1) as fw,
          tc.tile_pool(name="fin", bufs=1) as fn_,
          tc.tile_pool(name="psF", bufs=2, space="PSUM") as psf,
          tc.tile_pool(name="psO", bufs=1, space="PSUM") as pso):
        ww = fw.tile([P, KC, H], BF16)
        nc.sync.dma_start(out=ww[:], in_=D["W_w"][:])
        wp8 = fw.tile([P, 8, H], BF16)
        nc.sync.dma_start(out=wp8[:], in_=D["W_p"][:])
        ws8 = fw.tile([P, 8, H], BF16)
        nc.sync.dma_start(out=ws8[:], in_=D["W_s"][:])
        wfc1 = fw.tile([P, KC, 2 * H], BF16)
        nc.sync.dma_start(out=wfc1[:], in_=D["W_fc1"][:])
        wfc = fw.tile([P, 8, C], BF16)
        nc.sync.dma_start(out=wfc[:], in_=D["W_fc"][:])
        bfc_row = fw.tile([1, C], BF16)
        nc.sync.dma_start(out=bfc_row[:], in_=D["b_fc_row"][:])

        us = []
        for c in range(3):
            ut = fn_.tile([P, KC, BL], F32, tag=f"u{c}", name=f"u{c}")
            nc.vector.tensor_tensor(
                out=ut[:],
                in0=qfeat.rearrange("p k (b c) -> p k b c", c=3)[:, :, :, c],
                in1=vfeat.rearrange("p k (b c) -> p k b c", c=3)[:, :, :, c],
                op=ALU.add)
            ur = fn_.tile([P, KC, BL], BF16, tag=f"ur{c}", name=f"ur{c}")
            nc.scalar.copy(ur[:], ut[:])
            us.append(ur)

        def mlp(w8, nk, rhs_fn, bname, act, out_nch):
            ot = fn_.tile([P, out_nch, BL], BF16, tag=f"o{bname}", name=f"o{bname}")
            for m in range(out_nch):
                pt = psf.tile([P, BL], F32, tag="f")
                for k in range(nk):
                    nc.tensor.matmul(out=pt[:], lhsT=w8[:, k, m * P:(m + 1) * P],
                                     rhs=rhs_fn(k), start=(k == 0), stop=(k == nk - 1))
                nc.scalar.activation(ot[:, m, :], pt[:], act,
                                     bias=bias[bname][:, m:m + 1], scale=1.0)
            return ot

        hw = mlp(ww, KC, lambda k: us[0][:, k, :], "b_w", AF.Tanh, KC)
        hp = mlp(wp8, 8, lambda k: us[1][:, k, :] if k < KC else hw[:, k - KC, :],
                 "b_p", AF.Tanh, KC)
        hs = mlp(ws8, 8, lambda k: us[2][:, k, :] if k < KC else hp[:, k - KC, :],
                 "b_s", AF.Tanh, KC)
        f1 = mlp(wfc1, KC, lambda k: hs[:, k, :], "b_fc1", AF.Relu, 8)
        tap("hs", hs[:], [P, KC, BL], BF16)

        po = pso.tile([32, 2, 512], F32, tag="o")
        for nh in range(2):
            nw = 500
            nc.tensor.matmul(out=po[:, nh, :nw], lhsT=ones_bf[0:1, :BL],
                             rhs=bfc_row[0:1, nh * nw:(nh + 1) * nw],
                             start=True, stop=False)
            for k in range(8):
                nc.tensor.matmul(out=po[:, nh, :nw], lhsT=f1[:, k, :],
                                 rhs=wfc[:, k, nh * nw:(nh + 1) * nw],
                                 start=False, stop=(k == 7))
        mxt = fn_.tile([32, 1], F32, tag="mx")
        nc.vector.reduce_max(mxt[:], po[:, :, :500], axis=AX.XY)
        nmx = fn_.tile([32, 1], F32, tag="nmx")
        nc.vector.tensor_scalar_mul(nmx[:], mxt[:], -1.0)
        ext = fn_.tile([32, 2, 512], F32, tag="ext")
        nc.scalar.activation(ext[:, :, :500], po[:, :, :500], AF.Exp,
                             bias=nmx[:, 0:1], scale=1.0)
        smt = fn_.tile([32, 1], F32, tag="sm")
        nc.vector.reduce_sum(smt[:], ext[:, :, :500], axis=AX.XY)
        rct = fn_.tile([32, 1], F32, tag="rc")
        nc.vector.reciprocal(rct[:], smt[:])
        ot = fn_.tile([32, 2, 512], F32, tag="ot")
        nc.vector.tensor_scalar_mul(ot[:, :, :500], ext[:, :, :500], rct[:, 0:1])
        nc.sync.dma_start(out=D["yout"][:].rearrange("b (h n) -> b h n", n=500),
                          in_=ot[:, :, :500])


# ----------------------------------------------------------------------------
_NC_CACHE = {}


def get_nc():
    if "nc" not in _NC_CACHE:
        _NC_CACHE["nc"] = build_nc()
    return _NC_CACHE["nc"]


def run(inputs, trace=False, tmpdir=None):
    nc = get_nc()
    sh, percore = host_prep(inputs)
    in_maps = [{**sh, **pc} for pc in percore]
    res = run_bass_kernel_spmd(nc, in_maps, list(range(NCORES)), trace=trace,
                               tmpdir=tmpdir)
    out = np.concatenate([res.results[i]["yout"] for i in range(NCORES)], axis=0)
    return out, res


def kernel(**inputs):
    out, _ = run(inputs)
    return out.astype(np.float32)


# revision 8
# speedup vs baseline: 1.3299x; 1.3299x over previous
"""Trainium2 Bass kernel for nn_CoattentionModel (B,N,T,H,V,C = 256,196,26,512,12000,1000).

Data-parallel: 32 samples per core x 8 cores, weights replicated.
All GEMMs bf16 (fp32 PSUM accumulation). Feature-major activation layout
[128, chunk, cols]. Image col-tiled at 392 (=2*196) so co-attention quads read
whole tiles. v-attention runs with n on the free axis (wide matmul logits,
softmax on vector, weighted sums on the Pool engine).
"""
import numpy as np
import ml_dtypes

import concourse.bacc as bacc
import concourse.mybir as mybir
import concourse.tile as tile
from concourse import bass
from concourse.bass_utils import run_bass_kernel_spmd

B, NI, T, H, V, C = 256, 196, 26, 512, 12000, 1000
NCORES = 8
BL = B // NCORES          # 32
RI = BL * NI              # 6272
KC = 4                    # H/128
SW = 96                   # stacked width per sample (3 sources x 32)
QW = BL * SW              # 3072
WPW = 2 + BL * 32 + 2     # 1028
G4 = 4 * H                # 2048
RQ = BL * T               # 832
TW = 392                  # image col tile width (2 samples' worth of N)
NT = RI // TW             # 16 tiles

P = 128
F32 = mybir.dt.float32
F32R = mybir.dt.float32r
BF16 = mybir.dt.bfloat16
I32 = mybir.dt.int32
AF = mybir.ActivationFunctionType
ALU = mybir.AluOpType
AX = mybir.AxisListType
BF_NP = ml_dtypes.bfloat16

DEBUG_TAPS = ()


def _w4(w):  # [512, F] -> [128, 4, F] bf16
    return np.ascontiguousarray(
        np.asarray(w, np.float32).reshape(KC, P, -1).transpose(1, 0, 2)).astype(BF_NP)


def _b4(b, nch=KC):  # [nch*128] -> [128, nch] f32
    return np.ascontiguousarray(np.asarray(b, np.float32).reshape(nch, P).T)


def _bcast(b):  # [F] -> [128, F] bf16 broadcast over partitions
    return np.ascontiguousarray(
        np.broadcast_to(np.asarray(b, np.float32).reshape(1, -1), (P, len(np.asarray(b).reshape(-1))))
    ).astype(BF_NP)


def host_prep(inputs):
    f = {k: np.asarray(v) for k, v in inputs.items()}
    sh = {}
    sh["embed_table"] = np.ascontiguousarray(f["embed_table"], np.float32)
    for nm in ("W_img", "W_corr", "W_ie", "W_qe", "W_w"):
        sh[nm] = _w4(f[nm])
    sh["W_uni"] = _w4(f["W_uni"][0])
    sh["W_bi"] = np.stack([_w4(f["W_bi"][i]) for i in range(2)], 1)
    sh["W_tri"] = np.stack([_w4(f["W_tri"][i]) for i in range(3)], 1)
    perm = np.concatenate([np.arange(2 * H, 3 * H), np.arange(0, H),
                           np.arange(H, 2 * H), np.arange(3 * H, 4 * H)])
    sh["lstm_k"] = _w4(f["lstm_k"][:, perm])
    sh["lstm_rk"] = _w4(f["lstm_rk"][:, perm])
    sh["lstmb_row"] = f["lstm_b"][perm].astype(BF_NP).reshape(1, G4)
    sh["W_p"] = np.ascontiguousarray(
        f["W_p"].astype(np.float32).reshape(8, P, H).transpose(1, 0, 2)).astype(BF_NP)
    sh["W_s"] = np.ascontiguousarray(
        f["W_s"].astype(np.float32).reshape(8, P, H).transpose(1, 0, 2)).astype(BF_NP)
    sh["W_fc1"] = _w4(f["W_fc1"])
    sh["W_fc"] = np.ascontiguousarray(
        f["W_fc"].astype(np.float32).reshape(8, P, C).transpose(1, 0, 2)).astype(BF_NP)
    sh["w_qa"] = np.ascontiguousarray(_b4(f["w_qa"].reshape(-1)).astype(BF_NP))
    sh["w_ia"] = np.ascontiguousarray(_b4(f["w_ia"].reshape(-1)).astype(BF_NP))
    for nm in ("b_img", "b_corr", "b_ie", "b_qe", "b_uni", "b_bi", "b_tri",
               "b_w", "b_p", "b_s"):
        sh[nm] = _b4(f[nm])
    sh["b_fc1"] = _b4(f["b_fc1"], 8)
    sh["bie_row"] = f["b_ie"].astype(BF_NP).reshape(1, H)
    sh["bqe_row"] = f["b_qe"].astype(BF_NP).reshape(1, H)
    sh["b_fc_row"] = f["b_fc"].astype(BF_NP).reshape(1, C)
    sh["b_qa_col"] = np.full((1, 1), np.asarray(f["b_qa"]).reshape(-1)[0], np.float32)
    sh["b_ia_col"] = np.full((1, 1), np.asarray(f["b_ia"]).reshape(-1)[0], np.float32)
    sh["identity"] = np.eye(P, dtype=np.float32)
    sh["identity_bf"] = np.eye(P, dtype=BF_NP)
    sh["ones_bf"] = np.ones((P, 512), BF_NP)
    sh["zpad"] = np.zeros((P, KC, 2), np.float32)
    sh["zpad_bf"] = np.zeros((P, KC, 256), BF_NP)

    img = f["image"].astype(np.float32)
    qn = np.asarray(f["question"])
    percore = []
    for ci in range(NCORES):
        iloc = img[ci * BL:(ci + 1) * BL].reshape(RI, H)
        imageT = iloc.T.reshape(KC, P, RI).transpose(1, 0, 2)  # [P, KC, RI]
        imageT = np.ascontiguousarray(
            imageT.reshape(P, KC, NT, TW).transpose(2, 0, 1, 3)).astype(BF_NP)
        qi = np.full((BL, 32), V, np.int32)
        qi[:, :T] = qn[ci * BL:(ci + 1) * BL]
        qidx = np.ascontiguousarray(qi.reshape(8, P).T).astype(np.int32)
        percore.append({"imageT": imageT, "qidx": qidx})
    return sh, percore


# ----------------------------------------------------------------------------
def build_nc():
    nc = bacc.Bacc(name="coattn", num_swdge_queues=4)
    D = {}

    def dp(n, s, d=BF16):
        D[n] = nc.declare_dram_parameter(n, list(s), d, isOutput=False)

    dp("embed_table", [V, H], F32)
    for n in ("W_img", "W_corr", "W_ie", "W_qe", "W_w", "W_uni"):
        dp(n, [P, KC, H])
    dp("W_bi", [P, 2, KC, H])
    dp("W_tri", [P, 3, KC, H])
    dp("lstm_k", [P, KC, G4])
    dp("lstm_rk", [P, KC, G4])
    dp("lstmb_row", [1, G4])
    dp("W_p", [P, 8, H])
    dp("W_s", [P, 8, H])
    dp("W_fc1", [P, KC, 2 * H])
    dp("W_fc", [P, 8, C])
    dp("w_qa", [P, KC])
    dp("w_ia", [P, KC])
    for n in ("b_img", "b_corr", "b_ie", "b_qe", "b_uni", "b_bi", "b_tri",
              "b_w", "b_p", "b_s"):
        dp(n, [P, KC], F32)
    dp("b_fc1", [P, 8], F32)
    dp("bie_row", [1, H])
    dp("bqe_row", [1, H])
    dp("b_fc_row", [1, C])
    dp("b_qa_col", [1, 1], F32)
    dp("b_ia_col", [1, 1], F32)
    dp("identity", [P, P], F32R)
    dp("identity_bf", [P, P])
    dp("ones_bf", [P, 512])
    dp("zpad", [P, KC, 2], F32R)
    dp("zpad_bf", [P, KC, 256])
    dp("imageT", [NT, P, KC, TW])
    dp("qidx", [P, 8], I32)
    D["yout"] = nc.declare_dram_parameter("yout", [BL, C], F32, isOutput=True)

    D["corrT_d"] = nc.dram_tensor("corrT_d", [NT, P, KC, TW], BF16)
    D["embT_d"] = nc.dram_tensor("embT_d", [NT, P, KC, TW], BF16)
    D["rowemb_d"] = nc.dram_tensor("rowemb_d", [RI, H], BF16)
    D["qrow_d"] = nc.dram_tensor("qrow_d", [BL, P, H], BF16)
    D["xp_d"] = nc.dram_tensor("xp_d", [RQ, G4], BF16)

    def tap(name, ap, shape, dtype):
        if name in DEBUG_TAPS:
            o = nc.declare_dram_parameter("tap_" + name, list(shape), dtype, isOutput=True)
            nc.sync.dma_start(out=o[:], in_=ap)

    with tile.TileContext(nc) as tc:
        import contextlib
        with contextlib.ExitStack() as es:
            _build(nc, tc, D, tap, es)
    nc.compile()
    return nc


def _build(nc, tc, D, tap, es):
    import os as _os
    PH = int(_os.environ.get("KER_PHASES", "99"))
    import contextlib
    es_words = contextlib.ExitStack()
    es_phrase = contextlib.ExitStack()
    es_lk = contextlib.ExitStack()
    es_xp = contextlib.ExitStack()
    es_lstm = contextlib.ExitStack()
    es_sent = contextlib.ExitStack()
    es_qe = contextlib.ExitStack()
    const = es.enter_context(tc.tile_pool(name="const", bufs=1))
    resid = es.enter_context(tc.tile_pool(name="resid", bufs=1))

    def load_const(name, shape, dtype):
        t_ = const.tile(shape, dtype, tag=name, name="c_" + name)
        nc.sync.dma_start(out=t_[:], in_=D[name][:])
        return t_

    identR = load_const("identity", [P, P], F32R)
    identB = load_const("identity_bf", [P, P], BF16)
    ones_bf = load_const("ones_bf", [P, 512], BF16)
    zp = load_const("zpad", [P, KC, 2], F32R)
    zpb = load_const("zpad_bf", [P, KC, 256], BF16)
    bias = {n: load_const(n, [P, KC], F32) for n in
            ("b_img", "b_corr", "b_ie", "b_qe", "b_uni", "b_bi", "b_tri",
             "b_w", "b_p", "b_s")}
    bias["b_fc1"] = load_const("b_fc1", [P, 8], F32)
    bie_row = load_const("bie_row", [1, H], BF16)
    bqe_row = load_const("bqe_row", [1, H], BF16)
    wqa = load_const("w_qa", [P, KC], BF16)
    wia = load_const("w_ia", [P, KC], BF16)
    bqa_col = load_const("b_qa_col", [1, 1], F32)
    bia_col = load_const("b_ia_col", [1, 1], F32)

    # image-phase pools created BEFORE lstm-era pools so their SBUF/PSUM
    # addresses don't alias (address-reuse would serialize image behind LSTM)
    es_img = contextlib.ExitStack()
    iw = es_img.enter_context(tc.tile_pool(name="imgw", bufs=1))
    io2 = es_img.enter_context(tc.tile_pool(name="imgio2", bufs=2))
    io1 = es_img.enter_context(tc.tile_pool(name="imgio1", bufs=2))
    psa = es_img.enter_context(tc.tile_pool(name="psA", bufs=3, space="PSUM"))
    wimg = iw.tile([P, KC, H], BF16)
    nc.sync.dma_start(out=wimg[:], in_=D["W_img"][:])
    wcorr = iw.tile([P, KC, H], BF16)
    nc.sync.dma_start(out=wcorr[:], in_=D["W_corr"][:])
    wie = iw.tile([P, KC, H], BF16)
    nc.sync.dma_start(out=wie[:], in_=D["W_ie"][:])

    # long-lived phase pools, created in reverse order of release (LIFO)
    qepool = es_qe.enter_context(tc.tile_pool(name="qew", bufs=1))
    wqe = qepool.tile([P, KC, H], BF16)
    nc.sync.dma_start(out=wqe[:], in_=D["W_qe"][:])
    spool = es_sent.enter_context(tc.tile_pool(name="sentp", bufs=1))
    sentT = spool.tile([P, KC, RQ], BF16)
    xpool = es_xp.enter_context(tc.tile_pool(name="xpp", bufs=2))
    ppool = es_phrase.enter_context(tc.tile_pool(name="phrasep", bufs=1))
    phraseT = ppool.tile([P, KC, RQ], BF16)

    stk = resid.tile([P, KC, QW + 32], BF16)
    qembT = resid.tile([P, KC, QW], BF16)
    qfeat = resid.tile([P, KC, 3 * BL], F32)
    vfeat = resid.tile([P, KC, 3 * BL], F32)
    itf = resid.tile([P, KC, RI], BF16)      # tanh(image@W_img), feature-major
    stk_b = stk[:, :, :QW].rearrange("p k (b w) -> p k b w", w=SW)
    for c0 in range(3):
        for k in range(KC):
            nc.sync.dma_start(
                out=stk_b[:, k, :, 32 * c0 + 26:32 * c0 + 32],
                in_=zpb[:, k, :BL * 6].rearrange("p (b w) -> p b w", w=6))
    nc.sync.dma_start(out=stk[:, :, QW:], in_=zpb[:, :, :32])

    # ================= phase 0: embedding gather =================
    wpool = es_words.enter_context(tc.tile_pool(name="wordsp", bufs=1))
    wordsT = wpool.tile([P, KC, WPW], BF16)
    nc.sync.dma_start(out=wordsT[:, :, :2], in_=zpb[:, :, :2])
    nc.sync.dma_start(out=wordsT[:, :, WPW - 2:], in_=zpb[:, :, :2])
    qix = const.tile([P, 8], I32, tag="qix")
    nc.sync.dma_start(out=qix[:], in_=D["qidx"][:])
    with (tc.tile_pool(name="gath", bufs=4) as gp,
          tc.tile_pool(name="psG0", bufs=2, space="PSUM") as psg):
        for j in range(8):
            gt = gp.tile([P, H], F32R, tag="g")
            nc.vector.memset(gt[:].bitcast(F32), 0.0)
            nc.gpsimd.indirect_dma_start(
                out=gt[:], out_offset=None, in_=D["embed_table"][:],
                in_offset=bass.IndirectOffsetOnAxis(ap=qix[:, j:j + 1], axis=0),
                bounds_check=V - 1, oob_is_err=False)
            pt = psg.tile([P, H], F32R, tag="t")
            for k in range(KC):
                nc.tensor.transpose(out=pt[:, k * P:(k + 1) * P],
                                    in_=gt[:, k * P:(k + 1) * P], identity=identR[:])
            nc.scalar.copy(wordsT[:, :, 2 + j * P:2 + (j + 1) * P],
                           pt[:].rearrange("p (k c) -> p k c", c=P))
    wvalid = wordsT[:, :, 2:2 + BL * 32].rearrange("p k (b w) -> p k b w", w=32)[:, :, :, :T]
    nc.scalar.copy(stk_b[:, :, :, 0:T], wvalid)

    if PH < 1:
        return
    # ================= phase 1: convs -> phrase =================
    with (tc.tile_pool(name="convw", bufs=1) as cw,
          tc.tile_pool(name="convo", bufs=1) as co,
          tc.tile_pool(name="psC", bufs=3, space="PSUM") as psc):
        mx = co.tile([P, KC, RQ], BF16, tag="mx")
        cv1 = co.tile([P, KC, RQ], BF16, tag="cv1")
        cvs = [mx, cv1, cv1]
        taps_w = [
            ("b_uni", [(0, D["W_uni"][:])]),
            ("b_bi", [(-1, D["W_bi"][:, 0]), (1, D["W_bi"][:, 1])]),
            ("b_tri", [(-2, D["W_tri"][:, 0]), (0, D["W_tri"][:, 1]),
                       (2, D["W_tri"][:, 2])]),
        ]
        for ci, (bn, tp) in enumerate(taps_w):
            wts = []
            for ti, (shf, src) in enumerate(tp):
                wct = cw.tile([P, KC, H], BF16, tag=f"cw{ti}", name=f"cw{ti}")
                nc.sync.dma_start(out=wct[:], in_=src)
                wts.append((shf, wct))
            tp = [(shf, (lambda w_: lambda k, m: w_[:, k, m * P:(m + 1) * P])(w_))
                  for shf, w_ in wts]
            for hf in range(2):
                for m in range(KC):
                    pt = psc.tile([P, 16, T], F32, tag="c")
                    n_mm = len(tp) * KC
                    i_mm = 0
                    for shf, wfn in tp:
                        for k in range(KC):
                            rhs = wordsT[:, k, 2 + hf * 512 + shf:2 + hf * 512 + shf + 512] \
                                .rearrange("p (b w) -> p b w", w=32)[:, :, :T]
                            nc.tensor.matmul(out=pt[:], lhsT=wfn(k, m), rhs=rhs,
                                             start=(i_mm == 0), stop=(i_mm == n_mm - 1))
                            i_mm += 1
                    nc.scalar.activation(
                        cvs[ci][:, m, hf * 416:(hf + 1) * 416]
                        .rearrange("p (b w) -> p b w", w=T),
                        pt[:], AF.Tanh, bias=bias[bn][:, m:m + 1], scale=1.0)
                    if ci > 0:
                        nc.vector.tensor_tensor(
                            out=mx[:, m, hf * 416:(hf + 1) * 416],
                            in0=mx[:, m, hf * 416:(hf + 1) * 416],
                            in1=cv1[:, m, hf * 416:(hf + 1) * 416], op=ALU.max)
        nc.scalar.activation(phraseT[:], mx[:], AF.Tanh)
        nc.scalar.copy(stk_b[:, :, :, 32:32 + T],
                       phraseT.rearrange("p k (b t) -> p k b t", t=T))
    tap("stk", stk[:], [P, KC, QW + 32], BF16)

    # ================= phase 2: q_emb feature-major (3 sources) ==========
    if PH < 2:
        return
    def qemb_from(src_fn, block):
        # src_fn(k, hf) -> rhs AP [128, 16, 26] for batch-half hf, chunk k
        with tc.tile_pool(name=f"psQE{block}", bufs=2, space="PSUM") as psq_:
            for m in range(KC):
                for hf in range(2):
                    pt = psq_.tile([P, 16, T], F32, tag="q")
                    for k in range(KC):
                        nc.tensor.matmul(out=pt[:], lhsT=wqe[:, k, m * P:(m + 1) * P],
                                         rhs=src_fn(k, hf),
                                         start=(k == 0), stop=(k == KC - 1))
                    nc.vector.tensor_scalar_add(
                        qembT.rearrange("p k (b w) -> p k b w", w=SW)
                        [:, m, hf * 16:(hf + 1) * 16, 32 * block:32 * block + T],
                        pt[:], bias["b_qe"][:, m:m + 1])

    qemb_from(lambda k, hf: wordsT[:, k, 2 + hf * 512:2 + (hf + 1) * 512]
              .rearrange("p (b w) -> p b w", w=32)[:, :, :T], 0)
    es_words.close()
    qemb_from(lambda k, hf: phraseT[:, k, hf * 416:(hf + 1) * 416]
              .rearrange("p (b t) -> p b t", t=T), 1)

    if PH < 3:
        return
    # ================= phase 3: xp GEMM =================
    lkp = es_lk.enter_context(tc.tile_pool(name="lkp", bufs=2))
    lkc = es_lk.enter_context(tc.tile_pool(name="lkc", bufs=1))
    lstmb_row = lkc.tile([1, G4], BF16)
    nc.sync.dma_start(out=lstmb_row[:], in_=D["lstmb_row"][:])

    with tc.tile_pool(name="psX", bufs=2, space="PSUM") as psx:
        for j in range(KC):
            lk = lkp.tile([P, KC, 512], BF16, tag="lk")
            nc.sync.dma_start(out=lk[:], in_=D["lstm_k"][:, :, j * 512:(j + 1) * 512])
            for rc in range(7):           # b-major rows b*26+t, chunks of 128
                nr = 128 if rc < 6 else 64
                xstg = xpool.tile([P, 512], BF16, tag="xstg")
                pt = psx.tile([P, 512], F32, tag="x")
                nc.tensor.matmul(out=pt[:nr, :], lhsT=ones_bf[0:1, :nr],
                                 rhs=lstmb_row[0:1, j * 512:(j + 1) * 512],
                                 start=True, stop=False)
                for k in range(KC):
                    nc.tensor.matmul(out=pt[:nr, :],
                                     lhsT=phraseT[:, k, rc * P:rc * P + nr],
                                     rhs=lk[:, k, :],
                                     start=False, stop=(k == KC - 1))
                nc.vector.tensor_copy(xstg[:nr, :], pt[:nr, :])
                nc.sync.dma_start(
                    out=D["xp_d"][rc * P:rc * P + nr, j * 512:(j + 1) * 512],
                    in_=xstg[:nr, :])
    tap("xp", D["xp_d"][:], [RQ, G4], BF16)
    es_lk.close()
    es_phrase.close()

    if PH < 4:
        return
    # ================= phase 4: LSTM =================
    lrkp = es_lstm.enter_context(tc.tile_pool(name="lrkp", bufs=1))
    lrk = lrkp.tile([P, KC, G4], BF16)
    nc.sync.dma_start(out=lrk[:], in_=D["lstm_rk"][:])
    lpool = es_lstm.enter_context(tc.tile_pool(name="lstm", bufs=1))
    gs = lpool.tile([32, G4], F32)
    cst = lpool.tile([32, H], F32)
    tct = lpool.tile([32, H], F32)
    hrow = lpool.tile([32, H], BF16)
    hT_pool = es_lstm.enter_context(tc.tile_pool(name="hT", bufs=2))
    psg_l = es_lstm.enter_context(tc.tile_pool(name="psGate", bufs=3, space="PSUM"))
    psh_l = es_lstm.enter_context(tc.tile_pool(name="psH", bufs=2, space="PSUM"))
    hT_prev = None
    for t_ in range(T):
        xpt = hT_pool.tile([32, G4], BF16, tag="xpt")
        nc.sync.dma_start(out=xpt[:],
                          in_=D["xp_d"][:].rearrange("(b t) g -> t b g", t=T)[t_])
        for j in range(KC):
            pg = psg_l.tile([32, 512], F32, tag="g")
            nc.tensor.matmul(out=pg[:], lhsT=identB[0:32, 0:32],
                             rhs=xpt[:, j * 512:(j + 1) * 512],
                             start=True, stop=(hT_prev is None))
            if hT_prev is not None:
                for k in range(KC):
                    nc.tensor.matmul(out=pg[:],
                                     lhsT=hT_prev[:, k, :],
                                     rhs=lrk[:, k, j * 512:(j + 1) * 512],
                                     start=False, stop=(k == KC - 1))
            nc.scalar.activation(gs[:, j * 512:(j + 1) * 512], pg[:],
                                 AF.Tanh if j == 0 else AF.Sigmoid)
            # gate order: j0=g (tanh), j1=i, j2=f, j3=o
            if j == 1:
                igt = lpool.tile([32, H], F32, tag="igt", name="igt")
                nc.vector.tensor_tensor(out=igt[:], in0=gs[:, 512:1024],
                                        in1=gs[:, 0:512], op=ALU.mult)
            if j == 2 and t_ > 0:
                nc.vector.tensor_tensor(out=cst[:], in0=gs[:, 1024:1536],
                                        in1=cst[:], op=ALU.mult)
        if t_ == 0:
            nc.vector.tensor_copy(cst[:], igt[:])
        else:
            nc.vector.tensor_tensor(out=cst[:], in0=cst[:], in1=igt[:], op=ALU.add)
        nc.scalar.activation(tct[:], cst[:], AF.Tanh)
        nc.vector.tensor_tensor(out=hrow[:], in0=gs[:, 1536:2048], in1=tct[:],
                                op=ALU.mult)
        ph = psh_l.tile([P, P], BF16, tag="h")
        for k in range(KC):
            nc.tensor.transpose(out=ph[:, 32 * k:32 * (k + 1)],
                                in_=hrow[:, k * P:(k + 1) * P],
                                identity=identB[:32, :32])
        hT = hT_pool.tile([P, KC, 32], BF16, tag="hT")
        nc.scalar.copy(hT[:], ph[:].rearrange("p (k b) -> p k b", b=32))
        nc.scalar.copy(sentT.rearrange("p k (b t) -> p k b t", t=T)[:, :, :, t_], hT[:])
        hT_prev = hT
    tap("sentT", sentT[:], [P, KC, RQ], BF16)
    es_lstm.close()
    es_xp.close()

    if PH < 5:
        return
    # ============ phase 5: q_emb[sentence], stk[sentence], qrow ============
    nc.scalar.copy(stk_b[:, :, :, 64:64 + T],
                   sentT.rearrange("p k (b t) -> p k b t", t=T))
    qemb_from(lambda k, hf: sentT[:, k, hf * 416:(hf + 1) * 416]
              .rearrange("p (b t) -> p b t", t=T), 2)
    tap("qembT", qembT[:], [P, KC, QW], BF16)
    es_sent.close()

    with (tc.tile_pool(name="qrstage", bufs=2) as qrs,
          tc.tile_pool(name="psQR", bufs=2, space="PSUM") as psqr):
        for b in range(BL):
            pt = psqr.tile([96, H], F32, tag="r")
            nc.tensor.matmul(out=pt[:], lhsT=ones_bf[0:1, :96],
                             rhs=bqe_row[0:1, :], start=True, stop=False)
            for k in range(KC):
                nc.tensor.matmul(out=pt[:], lhsT=stk[:, k, b * SW:(b + 1) * SW],
                                 rhs=wqe[:, k, :], start=False, stop=(k == KC - 1))
            st = qrs.tile([96, H], BF16, tag="s")
            nc.vector.tensor_copy(st[:], pt[:])
            nc.sync.dma_start(out=D["qrow_d"][b, :96, :], in_=st[:])
    es_qe.close()

    if PH < 6:
        return
    # ================= phase 6: image GEMMs =================
    for ti in range(NT):
        ait = io2.tile([P, KC, TW], BF16, tag="ait")
        nc.sync.dma_start(out=ait[:], in_=D["imageT"][ti])
        for m in range(KC):
            pt = psa.tile([P, 512], F32, tag="m")
            for k in range(KC):
                nc.tensor.matmul(out=pt[:, :TW], lhsT=wimg[:, k, m * P:(m + 1) * P],
                                 rhs=ait[:, k, :], start=(k == 0), stop=(k == KC - 1))
            nc.scalar.activation(itf[:, m, ti * TW:(ti + 1) * TW], pt[:, :TW],
                                 AF.Tanh, bias=bias["b_img"][:, m:m + 1], scale=1.0)
        for wmat, bn, dst in ((wcorr, "b_corr", D["corrT_d"]),
                              (wie, "b_ie", D["embT_d"])):
            stg = io1.tile([P, KC, TW], BF16, tag="stg" + bn, name="stg" + bn)
            for m in range(KC):
                pt = psa.tile([P, 512], F32, tag="m")
                for k in range(KC):
                    nc.tensor.matmul(out=pt[:, :TW],
                                     lhsT=wmat[:, k, m * P:(m + 1) * P],
                                     rhs=itf[:, k, ti * TW:(ti + 1) * TW],
                                     start=(k == 0), stop=(k == KC - 1))
                nc.vector.tensor_scalar_add(stg[:, m, :], pt[:, :TW],
                                            bias[bn][:, m:m + 1])
            nc.sync.dma_start(out=dst[ti], in_=stg[:])
    # row-major img_embed for the ques_sum matmuls
    for rc in range(RI // P):
        pt = psa.tile([P, 512], F32, tag="m")
        nc.tensor.matmul(out=pt[:], lhsT=ones_bf[0:1, :P], rhs=bie_row[0:1, :],
                         start=True, stop=False)
        for k in range(KC):
            nc.tensor.matmul(out=pt[:], lhsT=itf[:, k, rc * P:(rc + 1) * P],
                             rhs=wie[:, k, :], start=False, stop=(k == KC - 1))
        rstg = io1.tile([P, H], BF16, tag="rstg")
        nc.vector.tensor_copy(rstg[:], pt[:])
        nc.sync.dma_start(out=D["rowemb_d"][rc * P:(rc + 1) * P, :], in_=rstg[:])
    es_img.close()
    tap("corrT", D["corrT_d"][:], [NT, P, KC, TW], BF16)
    tap("rowemb", D["rowemb_d"][:], [RI, H], BF16)

    if PH < 7:
        return
    # ================= phase 7: co-attention =================
    with (tc.tile_pool(name="castream", bufs=1) as cs,
          tc.tile_pool(name="castream2", bufs=2) as cs2,
          tc.tile_pool(name="camid", bufs=2) as cm,
          tc.tile_pool(name="cis", bufs=1) as cis,
          tc.tile_pool(name="caq", bufs=2) as cq,
          tc.tile_pool(name="cabig", bufs=1) as cbig,
          tc.tile_pool(name="psQ", bufs=2, space="PSUM") as psq,
          tc.tile_pool(name="psT", bufs=1, space="PSUM") as pst,
          tc.tile_pool(name="psIS", bufs=2, space="PSUM") as psis,
          tc.tile_pool(name="psV", bufs=2, space="PSUM") as psv):
        for qd in range(8):
            b0 = qd * 4
            corr_q = cs2.tile([P, KC, 4 * NI], BF16, tag="corr")
            emb_q = cs2.tile([P, KC, 4 * NI], BF16, tag="emb")
            for hq in range(2):
                nc.sync.dma_start(out=corr_q[:, :, hq * TW:(hq + 1) * TW],
                                  in_=D["corrT_d"][2 * qd + hq])
                nc.sync.dma_start(out=emb_q[:, :, hq * TW:(hq + 1) * TW],
                                  in_=D["embT_d"][2 * qd + hq])
            itq = itf[:, :, qd * 4 * NI:(qd + 1) * 4 * NI]
            qraw = cq.tile([P, KC, 12], F32, tag="qraw")
            vraw = cq.tile([P, KC, 12], F32, tag="vraw")
            qr = []
            rowe = []
            for s in range(4):
                b = b0 + s
                qr_ = cs.tile([96, H], BF16, tag=f"qr{s}", name=f"qr{s}")
                nc.sync.dma_start(out=qr_[:], in_=D["qrow_d"][b, :96, :])
                qr.append(qr_)
                re_ = cs.tile([P, 2, H], BF16, tag=f"re{s}", name=f"re{s}")
                nc.sync.dma_start(out=re_[:, 0, :], in_=D["rowemb_d"][b * NI:b * NI + P, :])
                nc.sync.dma_start(out=re_[:68, 1, :],
                                  in_=D["rowemb_d"][b * NI + P:(b + 1) * NI, :])
                rowe.append(re_)
            # --- wmT [196p, 96]/sample, quad-wide psums
            pw0 = psq.tile([P, 4 * SW], F32, tag="q")
            pw1 = psq.tile([68, 4 * SW], F32, tag="q")
            for s in range(4):
                b = b0 + s
                for hh, pw, pn in ((0, pw0, P), (1, pw1, 68)):
                    for k in range(KC):
                        nc.tensor.matmul(
                            out=pw[:pn, s * SW:(s + 1) * SW],
                            lhsT=corr_q[:, k, s * NI + hh * P:s * NI + hh * P + pn],
                            rhs=stk[:, k, b * SW:(b + 1) * SW],
                            start=(k == 0), stop=(k == KC - 1))
            wmT = cm.tile([P, 2, 4 * SW], BF16, tag="wmT")
            nc.scalar.activation(wmT[:, 0, :], pw0[:], AF.Tanh)
            nc.scalar.activation(wmT[:68, 1, :], pw1[:], AF.Tanh)
            # --- wm [96p(32c+t), 196]/sample via PE transpose
            wm = cm.tile([96, 4, NI], BF16, tag="wm")
            for pr in range(2):
                ptr = pst.tile([96, 2 * NI], BF16, tag="t")
                for si in range(2):
                    s = pr * 2 + si
                    for hh, pn in ((0, P), (1, 68)):
                        nc.tensor.transpose(
                            out=ptr[:, si * NI + hh * P:si * NI + hh * P + pn],
                            in_=wmT[:pn, hh, s * SW:(s + 1) * SW],
                            identity=identB[:pn, :pn])
                nc.vector.tensor_copy(wm[:, 2 * pr:2 * pr + 2, :],
                                      ptr[:].rearrange("p (s n) -> p s n", n=NI))
            # --- ques_sum feature-major (quad-wide)
            qsT = cm.tile([P, KC, 4 * SW], BF16, tag="qsT")
            for m in range(KC):
                pqs = psq.tile([P, 4 * SW], F32, tag="q")
                for s in range(4):
                    nc.tensor.matmul(out=pqs[:, s * SW:(s + 1) * SW],
                                     lhsT=rowe[s][:, 0, m * P:(m + 1) * P],
                                     rhs=wmT[:, 0, s * SW:(s + 1) * SW],
                                     start=True, stop=False)
                    nc.tensor.matmul(out=pqs[:, s * SW:(s + 1) * SW],
                                     lhsT=rowe[s][:68, 1, m * P:(m + 1) * P],
                                     rhs=wmT[:68, 1, s * SW:(s + 1) * SW],
                                     start=False, stop=True)
                nc.vector.tensor_tensor(out=pqs[:], in0=pqs[:],
                                        in1=qembT[:, m, b0 * SW:(b0 + 4) * SW],
                                        op=ALU.add)
                nc.scalar.activation(qsT[:, m, :], pqs[:], AF.Tanh)
            if qd == 0:
                tap("qsT0", qsT[:], [P, KC, 4 * SW], BF16)
            # --- q attention + q_feat
            pql = psv.tile([P, 512], F32, tag="v")
            for k in range(KC):
                nc.tensor.matmul(out=pql[0:1, 0:4 * SW], lhsT=wqa[:, k:k + 1],
                                 rhs=qsT[:, k, :], start=(k == 0), stop=(k == KC - 1))
            eqb = cq.tile([1, 4 * SW], BF16, tag="eqb")
            nc.scalar.activation(eqb[:], pql[0:1, 0:4 * SW], AF.Exp,
                                 bias=bqa_col[:, 0:1], scale=1.0)
            sq = cq.tile([1, 12], F32, tag="sq")
            nc.vector.reduce_sum(sq[:], eqb.rearrange("o (s w) -> o s w", w=32)[:, :, :T],
                                 axis=AX.X)
            pqb = psv.tile([P, 512], F32, tag="v")
            nc.tensor.matmul(out=pqb[:, 0:4 * SW], lhsT=ones_bf[0:1, :P], rhs=eqb[:],
                             start=True, stop=True)
            for m in range(KC):
                tq = cq.tile([P, 4 * SW], F32, tag="tq")
                nc.vector.tensor_tensor(out=tq[:], in0=stk[:, m, b0 * SW:(b0 + 4) * SW],
                                        in1=pqb[:, 0:4 * SW], op=ALU.mult)
                nc.vector.reduce_sum(qraw[:, m, :],
                                     tq.rearrange("p (s w) -> p s w", w=32)[:, :, :T],
                                     axis=AX.X)
            # --- img_sum feature-major; v logits with n on the free axis
            isT = cis.tile([P, KC, 3, 4 * NI], BF16, tag="isT")
            for c in range(3):
                for m in range(KC):
                    for half in range(2):
                        pis = psis.tile([P, TW], F32, tag="is")
                        nc.tensor.matmul(
                            out=pis[:], lhsT=identB[:],
                            rhs=emb_q[:, m, half * TW:(half + 1) * TW],
                            start=True, stop=False)
                        for si in range(2):
                            s = half * 2 + si
                            nc.tensor.matmul(
                                out=pis[:, si * NI:(si + 1) * NI],
                                lhsT=qr[s][32 * c:32 * c + T, m * P:(m + 1) * P],
                                rhs=wm[32 * c:32 * c + T, s, :],
                                start=False, stop=(si == 1), tile_position=(32 * c, 0))
                        nc.scalar.activation(
                            isT[:, m, c, half * TW:(half + 1) * TW], pis[:], AF.Tanh)
            # logits [1, n] per (c, half); softmax along free axis
            evb = cbig.tile([1, 3, 4 * NI], BF16, tag="evb")
            for c in range(3):
                for half in range(2):
                    plg = psv.tile([P, 512], F32, tag="v")
                    for k in range(KC):
                        nc.tensor.matmul(out=plg[0:1, 0:TW], lhsT=wia[:, k:k + 1],
                                         rhs=isT[:, k, c, half * TW:(half + 1) * TW],
                                         start=(k == 0), stop=(k == KC - 1))
                    nc.scalar.activation(evb[:, c, half * TW:(half + 1) * TW],
                                         plg[0:1, 0:TW], AF.Exp,
                                         bias=bia_col[:, 0:1], scale=1.0)
            sv = cq.tile([1, 12], F32, tag="sv")
            nc.vector.reduce_sum(sv.rearrange("o (s c) -> o c s", c=3),
                                 evb.rearrange("o c (s n) -> o c s n", n=NI),
                                 axis=AX.X)
            attB = cbig.tile([P, 3, 4 * NI], BF16, tag="attB")
            for c in range(3):
                for half in range(2):
                    pab = psv.tile([P, 512], F32, tag="v")
                    nc.tensor.matmul(out=pab[:, 0:TW], lhsT=ones_bf[0:1, :P],
                                     rhs=evb[0:1, c, half * TW:(half + 1) * TW],
                                     start=True, stop=True)
                    nc.scalar.copy(attB[:, c, half * TW:(half + 1) * TW],
                                   pab[:, 0:TW])
            # v_feat: att-weighted sums of tanh-image, on the Pool engine
            for k in range(KC):
                for c in range(3):
                    vt = cq.tile([P, 4 * NI], BF16, tag="vt")
                    nc.vector.tensor_tensor(out=vt[:], in0=itq[:, k, :],
                                            in1=attB[:, c, :], op=ALU.mult)
                    nc.vector.reduce_sum(
                        vraw[:, k, :].rearrange("p (s c) -> p s c", c=3)[:, :, c],
                        vt.rearrange("p (s n) -> p s n", n=NI), axis=AX.X)
            # normalize both branches: recip rows broadcast, one mult per k
            rqv = cq.tile([1, 24], F32, tag="rqv")
            nc.vector.reciprocal(rqv[0:1, 0:12], sq[:])
            nc.vector.reciprocal(rqv[0:1, 12:24], sv[:])
            rqvb = cq.tile([1, 24], BF16, tag="rqvb")
            nc.scalar.copy(rqvb[:], rqv[:])
            prb = psv.tile([P, 512], F32, tag="v")
            nc.tensor.matmul(out=prb[:, 0:24], lhsT=ones_bf[0:1, :P], rhs=rqvb[:],
                             start=True, stop=True)
            for k in range(KC):
                nc.vector.tensor_tensor(out=qfeat[:, k, b0 * 3:(b0 + 4) * 3],
                                        in0=qraw[:, k, :], in1=prb[:, 0:12],
                                        op=ALU.mult)
                nc.vector.tensor_tensor(out=vfeat[:, k, b0 * 3:(b0 + 4) * 3],
                                        in0=vraw[:, k, :], in1=prb[:, 12:24],
                                        op=ALU.mult)
    tap("qfeat", qfeat[:], [P, KC, 3 * BL], F32)
    tap("vfeat", vfeat[:], [P, KC, 3 * BL], F32)

    if PH < 8:
        return
    # ================= phase 8: final MLP + softmax =================
    with (tc.tile_pool(name="finw", bufs=1) as fw,
          tc.tile_pool(name="fin", bufs=1) as fn_,
          tc.tile_pool(name="psF", bufs=2, space="PSUM") as psf,
          tc.tile_pool(name="psO", bufs=1, space="PSUM") as pso):
        ww = fw.tile([P, KC, H], BF16)
        nc.sync.dma_start(out=ww[:], in_=D["W_w"][:])
        wp8 = fw.tile([P, 8, H], BF16)
        nc.sync.dma_start(out=wp8[:], in_=D["W_p"][:])
        ws8 = fw.tile([P, 8, H], BF16)
        nc.sync.dma_start(out=ws8[:], in_=D["W_s"][:])
        wfc1 = fw.tile([P, KC, 2 * H], BF16)
        nc.sync.dma_start(out=wfc1[:], in_=D["W_fc1"][:])
        wfc = fw.tile([P, 8, C], BF16)
        nc.sync.dma_start(out=wfc[:], in_=D["W_fc"][:])
        bfc_row = fw.tile([1, C], BF16)
        nc.sync.dma_start(out=bfc_row[:], in_=D["b_fc_row"][:])

        us = []
        for c in range(3):
            ut = fn_.tile([P, KC, BL], F32, tag=f"u{c}", name=f"u{c}")
            nc.vector.tensor_tensor(
                out=ut[:],
                in0=qfeat.rearrange("p k (b c) -> p k b c", c=3)[:, :, :, c],
                in1=vfeat.rearrange("p k (b c) -> p k b c", c=3)[:, :, :, c],
                op=ALU.add)
            ur = fn_.tile([P, KC, BL], BF16, tag=f"ur{c}", name=f"ur{c}")
            nc.scalar.copy(ur[:], ut[:])
            us.append(ur)

        def mlp(w8, nk, rhs_fn, bname, act, out_nch):
            ot = fn_.tile([P, out_nch, BL], BF16, tag=f"o{bname}", name=f"o{bname}")
            for m in range(out_nch):
                pt = psf.tile([P, BL], F32, tag="f")
                for k in range(nk):
                    nc.tensor.matmul(out=pt[:], lhsT=w8[:, k, m * P:(m + 1) * P],
                                     rhs=rhs_fn(k), start=(k == 0), stop=(k == nk - 1))
                nc.scalar.activation(ot[:, m, :], pt[:], act,
                                     bias=bias[bname][:, m:m + 1], scale=1.0)
            return ot

        hw = mlp(ww, KC, lambda k: us[0][:, k, :], "b_w", AF.Tanh, KC)
        hp = mlp(wp8, 8, lambda k: us[1][:, k, :] if k < KC else hw[:, k - KC, :],
                 "b_p", AF.Tanh, KC)
        hs = mlp(ws8, 8, lambda k: us[2][:, k, :] if k < KC else hp[:, k - KC, :],
                 "b_s", AF.Tanh, KC)
        f1 = mlp(wfc1, KC, lambda k: hs[:, k, :], "b_fc1", AF.Relu, 8)
        tap("hs", hs[:], [P, KC, BL], BF16)

        po = pso.tile([32, 2, 512], F32, tag="o")
        for nh in range(2):
            nw = 500
            nc.tensor.matmul(out=po[:, nh, :nw], lhsT=ones_bf[0:1, :BL],
                             rhs=bfc_row[0:1, nh * nw:(nh + 1) * nw],
                             start=True, stop=False)
            for k in range(8):
                nc.tensor.matmul(out=po[:, nh, :nw], lhsT=f1[:, k, :],
                                 rhs=wfc[:, k, nh * nw:(nh + 1) * nw],
                                 start=False, stop=(k == 7))
        mxt = fn_.tile([32, 1], F32, tag="mx")
        nc.vector.reduce_max(mxt[:], po[:, :, :500], axis=AX.XY)
        nmx = fn_.tile([32, 1], F32, tag="nmx")
        nc.vector.tensor_scalar_mul(nmx[:], mxt[:], -1.0)
        ext = fn_.tile([32, 2, 512], F32, tag="ext")
        nc.scalar.activation(ext[:, :, :500], po[:, :, :500], AF.Exp,
                             bias=nmx[:, 0:1], scale=1.0)
        smt = fn_.tile([32, 1], F32, tag="sm")
        nc.vector.reduce_sum(smt[:], ext[:, :, :500], axis=AX.XY)
        rct = fn_.tile([32, 1], F32, tag="rc")
        nc.vector.reciprocal(rct[:], smt[:])
        ot = fn_.tile([32, 2, 512], F32, tag="ot")
        nc.vector.tensor_scalar_mul(ot[:, :, :500], ext[:, :, :500], rct[:, 0:1])
        nc.sync.dma_start(out=D["yout"][:].rearrange("b (h n) -> b h n", n=500),
                          in_=ot[:, :, :500])


# ----------------------------------------------------------------------------
_NC_CACHE = {}


def get_nc():
    if "nc" not in _NC_CACHE:
        _NC_CACHE["nc"] = build_nc()
    return _NC_CACHE["nc"]


def run(inputs, trace=False, tmpdir=None):
    nc = get_nc()
    sh, percore = host_prep(inputs)
    in_maps = [{**sh, **pc} for pc in percore]
    res = run_bass_kernel_spmd(nc, in_maps, list(range(NCORES)), trace=trace,
                               tmpdir=tmpdir)
    out = np.concatenate([res.results[i]["yout"] for i in range(NCORES)], axis=0)
    return out, res


def kernel(**inputs):
    out, _ = run(inputs)
    return out.astype(np.float32)
